# revision 28
# baseline (speedup 1.0000x reference)
"""RNN-T JointNet kernel for 8 Trainium2 NeuronCores.

Math: out[b,t,u,:] = gelu_tanh(concat(enc[b,t], dec[b,u])) @ W_fc^T + b_fc
Since gelu is elementwise, gelu(concat(a,b)) = concat(gelu(a), gelu(b)), so
  out[b,t,u,:] = P_enc[b,t,:] + P_dec[b,u,:]
with P_enc = gelu(enc) @ W_fc[:, :512]^T  (tiny matmul, (B,T,V))
     P_dec = gelu(dec) @ W_fc[:, 512:]^T + b_fc  (tiny matmul, (B,U,V))
The dominant cost is streaming the (B,T,U,V) output to HBM; tolerance is
2e-2 so the output is written as fp16 (rel err ~4e-4), halving HBM traffic
— the kernel is then HBM-store-bound (~20MB/core at ~340-360 GB/s).

Layout: V on SBUF partitions (V=640 = 5 chunks of 128), tables kept
TRANSPOSED: P_encT[v,t] ([128,300] fp16 per chunk), P_decT[v,u] ([128,51]
fp32 per chunk). For each u the [128 v, 300 t] output slab is ONE
tensor_scalar_add (DVE, ~293ns) / activation-Identity-bias (ACT, ~540ns)
with the per-partition scalar P_decT[:,u] — no PE broadcasts, no PSUM
round-trip. GpSimd's tensor_scalar is 15x slower than DVE; it is used
only to issue half the stores (SWDGE ring), keeping ACT free of DMA
issue overhead.

The host pre-transposes enc/dec to [D,T]/[D,U] fp16, so gelu applies
directly in the [d,*] layout and NO on-device transposes are needed.

Sharding: 8 cores = 4 batches x 2 u-halves (U=101 split 51+50, padded to
102). Device out is [V, UCORE, T] fp16; host transposes/casts on gather.

Per core: load encT/decT/W^T fp16 (v-chunk-0 weight columns first so the
first matmuls unblock early) -> 3 fused gelus (ACT) -> per v-chunk 4+4
K=128 fp16 matmuls -> P_encT (fp16) / P_decT (fp32, bias folded via K=1
ones matmul), pipelined one v-chunk ahead of the adds; the add stream
uses small tiles first (store stream starts ~15us in), then 26/25-row
tiles (~1.5-2MB per DMA, alternating sync-HWDGE / gpsimd-SWDGE rings).
"""

import numpy as np

B, T, U = 4, 300, 101
D = 512
V = 640
UCORE = 51  # u rows per core (U split 51 + 50, padded to 102)
NCORES = 8

LAST_RESULT = None  # BassKernelResults of the most recent run (for test.py)
RUN_KWARGS = {}  # extra kwargs test.py may inject (e.g. tmpdir for traces)

_cache = {}


def _build():
    import concourse.mybir as mybir
    from concourse import bacc
    from concourse.tile import TileContext

    f32 = mybir.dt.float32
    f16 = mybir.dt.float16
    AF = mybir.ActivationFunctionType

    nc = bacc.Bacc()
    enc_d = nc.dram_tensor("encT", [D, T], f16, kind="ExternalInput")
    dec_d = nc.dram_tensor("decT", [D, UCORE], f16, kind="ExternalInput")
    wT_d = nc.dram_tensor("wT", [2 * D, V], f16, kind="ExternalInput")
    bias_d = nc.dram_tensor("bias", [1, V], f16, kind="ExternalInput")
    out_d = nc.dram_tensor("out", [V, UCORE, T], f16, kind="ExternalOutput")

    with TileContext(nc) as tc:
        with (
            tc.tile_pool(name="const", bufs=1) as constp,
            tc.tile_pool(name="work", bufs=2) as work,
            tc.tile_pool(name="persist", bufs=1) as persist,
            tc.tile_pool(name="outp", bufs=2) as outp,
            tc.tile_pool(name="outp26", bufs=2) as outp26,
            tc.tile_pool(name="mm_psum", bufs=2, space="PSUM") as mm_psum,
            tc.tile_pool(name="pd_psum", bufs=2, space="PSUM") as pd_psum,
        ):
            # loads, head-critical first: enc (gates gelu->pe0 matmul, the
            # longest chain), dec, v-chunk-0 weight columns, bias; the
            # remaining weight columns (1MB) land last since v-chunks 1-4
            # aren't needed until ~25us in
            el = [work.tile([128, T], f16, tag=f"ld_enc{q}", name=f"el{q}") for q in range(4)]
            for q in range(4):
                # quarters alternate the two HWDGE rings so data moves in
                # parallel and each d-chunk's gelu+matmul unblocks as soon
                # as its own quarter lands
                (nc.sync if q % 2 == 0 else nc.scalar).dma_start(
                    el[q][:], enc_d[q * 128 : (q + 1) * 128, :]
                )
            dl = work.tile([128, 4, UCORE], f16, tag="ld_dec")
            nc.sync.dma_start(dl[:], dec_d.rearrange("(c p) u -> p c u", p=128))
            w0_sb = persist.tile([128, 8, 256], f16, tag="w0", name="w0")
            nc.sync.dma_start(
                w0_sb[:], wT_d[:, 0:256].rearrange("(kc p) v -> p kc v", p=128)
            )
            bias_sb = constp.tile([1, V], f16)
            nc.sync.dma_start(bias_sb[:], bias_d[:])
            wr_sb = persist.tile([128, 8, V - 256], f16, tag="wr", name="wr")
            nc.sync.dma_start(
                wr_sb[:], wT_d[:, 256:V].rearrange("(kc p) v -> p kc v", p=128)
            )

            def wslice(kc, vc):
                if vc < 2:
                    return w0_sb[:, kc, vc * 128 : (vc + 1) * 128]
                return wr_sb[:, kc, (vc - 2) * 128 : (vc - 1) * 128]

            ones = constp.tile([1, 64], f16)
            nc.vector.memset(ones[:], 1.0)

            # gelu in the [d, *] layout (elementwise, so transpose-free)
            genc = [persist.tile([128, 2, T], f16, tag=f"genc{h}", name=f"genc{h}") for h in range(2)]
            gdec = persist.tile([128, 4, UCORE], f16, tag="gdec")
            for q in range(4):
                nc.scalar.activation(genc[q // 2][:, q % 2, :], el[q][:], AF.Gelu_apprx_tanh)
            nc.scalar.activation(gdec[:], dl[:], AF.Gelu_apprx_tanh)

            # P tables: matmuls + copies for chunk vc+1 are emitted BEFORE
            # chunk vc's adds, so by the time the adders reach a chunk its
            # tables are already in SBUF (no boundary bubble in the store
            # stream). PE runs a full chunk ahead; psum pools hold 2.
            pd_sb = [persist.tile([128, UCORE], f32, tag=f"pd{vc}", name=f"pd{vc}") for vc in range(5)]
            pe_sb = [persist.tile([128, T], f16, tag=f"pe{vc}", name=f"pe{vc}") for vc in range(5)]

            def emit_tables(vc, pe_copy_eng):
                vsl = slice(vc * 128, (vc + 1) * 128)
                # P_encT[v,t] (fp16 in SBUF); first so chunk 0's longest
                # dependency (genc) is consumed as early as possible
                ps = mm_psum.tile([128, T], f32, tag="pemm")
                for dch in range(4):
                    nc.tensor.matmul(
                        ps[:, :],
                        wslice(dch, vc),
                        genc[dch // 2][:, dch % 2, :],
                        start=(dch == 0),
                        stop=(dch == 3),
                    )
                if pe_copy_eng is nc.vector:
                    nc.vector.tensor_copy(pe_sb[vc][:, :], ps[:, :])
                else:
                    nc.scalar.copy(pe_sb[vc][:, :], ps[:, :])
                # P_decT[v,u] (fp32, + bias via K=1 ones matmul)
                ps = pd_psum.tile([128, UCORE], f32, tag="pdmm")
                for dch in range(4):
                    nc.tensor.matmul(
                        ps[:, :],
                        wslice(4 + dch, vc),
                        gdec[:, dch, :],
                        start=(dch == 0),
                        stop=False,
                    )
                nc.tensor.matmul(
                    ps[:, :], bias_sb[:1, vsl], ones[:1, :UCORE], start=False, stop=True
                )
                nc.vector.tensor_copy(pd_sb[vc][:, :], ps[:, :])

            emit_tables(0, nc.vector)  # DVE is idle in the head; ACT has gelus
            # vc0: small tiles so the DMA-bound stream starts early; vc1-4:
            # two big tiles each (fewer DMAs + sem events). DVE takes ~2/3
            # of each tile's rows (~293ns/row vs ACT ~540ns/row). Stores
            # alternate the sync HWDGE ring and gpsimd SWDGE — gpsimd is
            # useless for math (its tensor_scalar is 15x slower than DVE)
            # but fine as a DMA issuer, keeping ACT free of issue overhead.
            dve_share = {6: 4, 7: 5, 13: 9, 12: 8, 26: 17, 25: 17}
            tile_no = 0
            for vc in range(5):
                blocks = [6, 7, 13, 13, 12] if vc == 0 else [26, 25]
                u0 = 0
                for bi, rn in enumerate(blocks):
                    # tables for vc+1 go before the last two blocks' adds:
                    # their matmuls are long done on PE by then, so the
                    # copies can't stall the adders
                    if bi == len(blocks) - 2 and vc + 1 < 5:
                        emit_tables(vc + 1, nc.scalar)
                    pool = outp if rn <= 13 else outp26
                    ot = pool.tile([128, rn, T], f16, tag=f"ot{rn}", name="ot")
                    for j in range(rn):
                        sc = pd_sb[vc][:, u0 + j : u0 + j + 1]
                        if j < dve_share[rn]:
                            nc.vector.tensor_scalar_add(ot[:, j, :], pe_sb[vc][:, :], sc)
                        else:
                            nc.scalar.activation(
                                ot[:, j, :], pe_sb[vc][:, :], AF.Identity, bias=sc
                            )
                    eng = nc.sync if tile_no % 2 == 0 else nc.gpsimd
                    tile_no += 1
                    eng.dma_start(
                        out_d[vc * 128 : (vc + 1) * 128, u0 : u0 + rn, :], ot[:, :, :]
                    )
                    u0 += rn

    nc.compile()
    return nc


def kernel(encoder_outputs, decoder_outputs, W_fc, b_fc):
    global LAST_RESULT
    import os

    from concourse.bass_utils import run_bass_kernel_spmd

    enc = np.asarray(encoder_outputs, dtype=np.float32)
    dec = np.asarray(decoder_outputs, dtype=np.float32)
    wT = np.ascontiguousarray(np.asarray(W_fc, dtype=np.float32).T.astype(np.float16))
    bias = np.ascontiguousarray(np.asarray(b_fc, dtype=np.float16)[None, :])

    dec_pad = np.zeros((B, 2 * UCORE, D), dtype=np.float32)
    dec_pad[:, :U, :] = dec

    if "nc" not in _cache:
        _cache["nc"] = _build()
    nc = _cache["nc"]

    in_maps = []
    for c in range(NCORES):
        b, uh = c // 2, c % 2
        in_maps.append(
            {
                "encT": np.ascontiguousarray(enc[b].T.astype(np.float16)),
                "decT": np.ascontiguousarray(
                    dec_pad[b, uh * UCORE : (uh + 1) * UCORE].T.astype(np.float16)
                ),
                "wT": wT,
                "bias": bias,
            }
        )

    res = run_bass_kernel_spmd(
        nc,
        in_maps,
        list(range(NCORES)),
        trace=bool(int(os.environ.get("KJ_TRACE", "0"))),
        **RUN_KWARGS,
    )
    LAST_RESULT = res

    out = np.empty((B, T, U, V), dtype=np.float32)
    for c in range(NCORES):
        b, uh = c // 2, c % 2
        cut = res.results[c]["out"]  # (640, 52, 300) fp16
        n = UCORE if uh == 0 else U - UCORE  # 51 / 50
        out[b, :, uh * UCORE : uh * UCORE + n, :] = (
            cut[:, :n, :].astype(np.float32).transpose(2, 1, 0)
        )
    return out
